# revision 23
# baseline (speedup 1.0000x reference)
"""Distributed Trainium2 kernel for nn_Attention_37958920962105.

GQA attention layer (DIM=4096, 32 q heads, 8 kv heads, head_dim=128,
B=2, S=2048) sharded tensor-parallel over GQA groups across 8 cores:
core c owns q heads 4c..4c+3 and kv head c.  Per core:
  1. QKV projection (transposed layouts) + RoPE + v transpose
  2. attention: scores (2 kt per PSUM pair-tile) -> one mega-exp ACT per
     pair -> AV matmuls; softmax denominators via DVE quad-tree sums +
     one ones-matmul per 4 kt tiles (cuts PE streaming 4x)
  3. AllToAll (one per local head) to token-shard y
  4. out projection on this core's 512-token chunk
Output chunks are reassembled on the host.
"""

import math
import sys
import types
from contextlib import ExitStack

import numpy as np
import ml_dtypes

import concourse.bass as bass
import concourse.mybir as mybir
import concourse.tile as tile
from concourse import bacc
from concourse.bass_utils import run_bass_kernel_spmd

BF = mybir.dt.bfloat16
F32 = mybir.dt.float32
bf16 = ml_dtypes.bfloat16

P = 128
DIM = 4096
N_HEAD = 32
N_KV = 8
HD = 128
B = 2
S = 2048
TOK = B * S          # 4096
NCORES = 8
HPC = N_HEAD // N_KV  # 4 q heads per core
FQKV = (HPC + 2) * HD  # 768 qkv rows per core
KC = DIM // P        # 32 contraction chunks
N_TT = TOK // 512    # 8 token tiles of 512
QT_N = S // 512      # 4 q tiles per batch
KT_N = S // P        # 16 k tiles per batch
SCALE = 1.0 / math.sqrt(HD)


def _install_profile_shim():
    if 'antenv.axon_hooks' in sys.modules:
        return
    try:
        from trn_agent_boot.trn_boot import _ntff_profile_via_ctypes
        hook = _ntff_profile_via_ctypes('/opt/axon/libaxon_pjrt.so')
    except Exception:
        hook = None
    mod = types.ModuleType('antenv.axon_hooks')
    mod._hook = hook
    mod.get_axon_ntff_profile_hook = lambda: mod._hook
    mod.set_axon_ntff_profile_hook = lambda h: setattr(mod, '_hook', h)
    sys.modules['antenv.axon_hooks'] = mod
    try:
        import antenv
        antenv.axon_hooks = mod
    except ImportError:
        pass


# ---------------------------------------------------------------------------
# host-side prep
# ---------------------------------------------------------------------------

def _classify_mask(mask):
    """mask: [S(q), S(k)] bool.  Returns (cls, mask_tiles) where
    cls[qt][kt] in {'skip', 'free', ('diag', off), int mask-tile-index};
    mask tiles are transposed [128 k, 512 q] bf16."""
    cls = [[None] * KT_N for _ in range(QT_N)]
    tiles = []
    seen = {}
    qi = np.arange(512)[:, None]
    ki = np.arange(P)[None, :]
    for qt in range(QT_N):
        for kt in range(KT_N):
            blk = mask[qt * 512:(qt + 1) * 512, kt * P:(kt + 1) * P]
            if not blk.any():
                cls[qt][kt] = 'skip'
                continue
            if blk.all():
                cls[qt][kt] = 'free'
                continue
            off = kt * P - qt * 512
            if 0 <= off < 512 and np.array_equal(blk, (off + ki) <= qi):
                cls[qt][kt] = ('diag', off)
                continue
            key = blk.tobytes()
            if key not in seen:
                seen[key] = len(tiles)
                tiles.append(np.ascontiguousarray(blk.T).astype(bf16))
            cls[qt][kt] = seen[key]
    return cls, tiles


def _prep(x, freqs_cis, mask_cache, wqkv, wo):
    x = np.asarray(x, dtype=np.float32)
    freqs_cis = np.asarray(freqs_cis, dtype=np.float32)
    wqkv = np.asarray(wqkv, dtype=np.float32)
    wo = np.asarray(wo, dtype=np.float32)
    mask = np.asarray(mask_cache)[0, 0]

    xT = np.ascontiguousarray(x.reshape(TOK, DIM).T).astype(bf16)

    wTs = []
    for c in range(NCORES):
        w_c = np.concatenate([
            wqkv[HPC * HD * c: HPC * HD * (c + 1)],          # 4 q heads
            wqkv[N_HEAD * HD + HD * c: N_HEAD * HD + HD * (c + 1)],   # k head
            wqkv[(N_HEAD + N_KV) * HD + HD * c:
                 (N_HEAD + N_KV) * HD + HD * (c + 1)],       # v head
        ], axis=0)                                           # [768, DIM]
        wTs.append(np.ascontiguousarray(w_c.T).astype(bf16))  # [DIM, 768]

    # wo permuted so row-block dbi = m*8 + cc holds global head 4*cc + m
    woT = np.ascontiguousarray(wo.T)                 # [d, o]
    woT_h = woT.reshape(N_HEAD, HD, DIM)
    perm = [4 * (dbi % NCORES) + dbi // NCORES for dbi in range(N_HEAD)]
    woT_perm = np.ascontiguousarray(woT_h[perm].reshape(DIM, DIM)).astype(bf16)

    f0 = freqs_cis[:, :, 0].T                        # [64, S]
    f1 = freqs_cis[:, :, 1].T
    ropeA = np.repeat(f0, 2, axis=0).astype(bf16)    # [128, S]
    ropeB = np.empty((HD, S), dtype=np.float32)
    ropeB[0::2] = -f1
    ropeB[1::2] = f1
    ropeB = ropeB.astype(bf16)

    pswap = np.zeros((P, P), dtype=bf16)
    for i in range(P):
        pswap[i, i ^ 1] = 1
    ident = np.eye(P, dtype=bf16)
    ones_col = np.ones((P, 1), dtype=bf16)

    tri = (np.arange(P)[:, None] <= np.arange(P)[None, :]).astype(bf16)
    cls, mask_tiles = _classify_mask(mask)
    masks = (np.concatenate([t for t in mask_tiles], axis=0)
             if mask_tiles else None)                # [n*128, 512] bf16

    return dict(xT=xT, wTs=wTs, woT=woT_perm, ropeA=ropeA, ropeB=ropeB,
                pswap=pswap, ident=ident, ones=ones_col,
                tri=tri, cls=cls, masks=masks)


# ---------------------------------------------------------------------------
# device kernel builder
# ---------------------------------------------------------------------------

def _build(cls, n_masks, debug=False):
    nc = bacc.Bacc("TRN2", target_bir_lowering=False, debug=False,
                   num_devices=NCORES)
    xT_d = nc.dram_tensor("xT", [DIM, TOK], BF, kind="ExternalInput")
    wT_d = nc.dram_tensor("wT", [DIM, FQKV], BF, kind="ExternalInput")
    woT_d = nc.dram_tensor("woT", [DIM, DIM], BF, kind="ExternalInput")
    ropeA_d = nc.dram_tensor("ropeA", [P, S], BF, kind="ExternalInput")
    ropeB_d = nc.dram_tensor("ropeB", [P, S], BF, kind="ExternalInput")
    pswap_d = nc.dram_tensor("pswap", [P, P], BF, kind="ExternalInput")
    ident_d = nc.dram_tensor("ident", [P, P], BF, kind="ExternalInput")
    ones_d = nc.dram_tensor("ones", [P, 1], BF, kind="ExternalInput")
    tri_d = nc.dram_tensor("tri", [P, P], BF, kind="ExternalInput")
    masks_d = (nc.dram_tensor("masks", [n_masks * P, 512], BF,
                              kind="ExternalInput") if n_masks else None)
    out_d = nc.dram_tensor("out", [512, DIM], F32, kind="ExternalOutput")

    EXP = mybir.ActivationFunctionType.Exp
    rg = [list(range(NCORES))]

    wT_r = wT_d[:, :].rearrange("(k p) f -> p k f", p=P)      # [P, KC, FQKV]
    xT_r = xT_d[:, :].rearrange("(k p) t -> p k t", p=P)      # [P, KC, TOK]
    woT_r = woT_d[:, :].rearrange("(k p) o -> p k o", p=P)    # [P, KC, DIM]

    with tile.TileContext(nc) as tc:
        with ExitStack() as top:
            const = top.enter_context(tc.tile_pool(name="const", bufs=1))
            acts = top.enter_context(tc.tile_pool(name="acts", bufs=1))
            dramp = top.enter_context(tc.tile_pool(name="dramp", bufs=1,
                                                   space="DRAM"))

            qT = [acts.tile([P, TOK], BF, name=f"qT{h}") for h in range(HPC)]
            kT = acts.tile([P, TOK], BF, name="kT")
            vv = [acts.tile([P, P], BF, name=f"v{i}") for i in range(TOK // P)]

            a2a_in = [dramp.tile([NCORES * P, 512], BF, name=f"a2ai{m}")
                      for m in range(HPC)]
            a2a_out = [dramp.tile([NCORES * P, 512], BF, name=f"a2ao{m}")
                       for m in range(HPC)]
            # tiny dummy A2A issued during ph1: absorbs the ~14us
            # first-collective setup so the four real (serialized) A2As
            # all complete earlier relative to ph3's yfm reads
            warm_i = dramp.tile([NCORES, 512], BF, name="warm_i")
            warm_o = dramp.tile([NCORES, 512], BF, name="warm_o")

            # ---------------- PE warmup during initial DMA wait ----------
            # junk lives in the never-freed const pool: if it sat in a
            # scratch pool, the first wqkv DMA would land on its address
            # and wait (WAR) for every warmup matmul before loading
            with ExitStack() as wu, nc.named_scope("warmup"):
                wups = wu.enter_context(tc.tile_pool(name="wups", bufs=1,
                                                     space="PSUM"))
                junk = const.tile([P, 256], BF, name="junk")
                nc.vector.memset(junk[:], 0)
                wps = wups.tile([P, 256], F32, name="wps")
                for _ in range(8):
                    nc.tensor.matmul(wps[:], junk[:, :P], junk[:],
                                     start=True, stop=True,
                                     skip_group_check=True)

            # ---------------- phase 1: QKV + rope + v transpose ----------
            with ExitStack() as ph1, nc.named_scope("ph1_qkv"):
                wp = ph1.enter_context(tc.tile_pool(name="wp", bufs=1))
                xp = ph1.enter_context(tc.tile_pool(name="xp", bufs=6))
                xp0 = ph1.enter_context(tc.tile_pool(name="xp0", bufs=16))

                def load_x(tt):
                    tiles = []
                    for g in range(4):
                        t = xp.tile([P, 8, 512], BF, name="xt8")
                        (nc.scalar if g % 2 else nc.sync).dma_start(
                            t[:], xT_r[:, g * 8:(g + 1) * 8,
                                       tt * 512:(tt + 1) * 512])
                        tiles.append(t)
                    return tiles

                wtiles = []
                xtiles0 = []

                def load_group(g):
                    # per-kc granularity in consumption order: startup is
                    # HBM-bound with all 8 cores racing, so the first MM's
                    # deps must head their queues.  g0 w on sync (HWDGE,
                    # fastest first-byte) with the f0 32KB slice split out,
                    # x0 + g1 on scalar, g2/g3 ahead of consts on gpsimd
                    if g == 0:
                        ew, ex = nc.sync, nc.scalar
                    elif g == 1:
                        ew = ex = nc.scalar
                    elif g < 4:
                        ew = ex = nc.gpsimd
                    else:
                        ew, ex = ((nc.sync, nc.scalar) if g % 2 else
                                  (nc.scalar, nc.sync))
                    w2 = wp.tile([P, 2, FQKV], BF, name=f"w2_{g}")
                    x2 = xp0.tile([P, 2, 512], BF, name="x2")
                    for j in range(2):
                        if g == 0 and j == 0:
                            ew.dma_start(w2[:, j, 0:P],
                                         wT_r[:, g * 2 + j, 0:P])
                            ew.dma_start(w2[:, j, P:],
                                         wT_r[:, g * 2 + j, P:])
                        else:
                            ew.dma_start(w2[:, j, :],
                                         wT_r[:, g * 2 + j, :])
                        ex.dma_start(x2[:, j, :],
                                     xT_r[:, g * 2 + j, 0:512])
                    wtiles.append(w2)
                    xtiles0.append(x2)

                qkvp = ph1.enter_context(
                    tc.tile_pool(name="qkvp", bufs=6, space="PSUM"))
                miscp = ph1.enter_context(
                    tc.tile_pool(name="miscp", bufs=2, space="PSUM"))
                stg = ph1.enter_context(tc.tile_pool(name="stg", bufs=5))

                for g in range(4):
                    load_group(g)

                nc.gpsimd.collective_compute(
                    "AllToAll", mybir.AluOpType.bypass,
                    replica_groups=rg,
                    ins=[warm_i.opt()], outs=[warm_o.opt()])

                ropeA_sb = const.tile([P, S], BF, name="ropeA_sb")
                nc.gpsimd.dma_start(ropeA_sb[:], ropeA_d[:])
                ropeB_sb = const.tile([P, S], BF, name="ropeB_sb")
                nc.gpsimd.dma_start(ropeB_sb[:], ropeB_d[:])
                pswap_sb = const.tile([P, P], BF, name="pswap_sb")
                nc.gpsimd.dma_start(pswap_sb[:], pswap_d[:])
                ident_sb = const.tile([P, P], BF, name="ident_sb")
                nc.gpsimd.dma_start(ident_sb[:], ident_d[:])
                ones_sb = const.tile([P, 1], BF, name="ones_sb")
                nc.gpsimd.dma_start(ones_sb[:], ones_d[:])
                tri_sb = const.tile([P, P], BF, name="tri_sb")
                nc.gpsimd.dma_start(tri_sb[:], tri_d[:])
                mask_sb = []
                for i in range(n_masks):
                    m = const.tile([P, 512], BF, name=f"mask{i}")
                    nc.gpsimd.dma_start(m[:], masks_d[i * P:(i + 1) * P, :])
                    mask_sb.append(m)

                def w_of(kc, f):
                    return wtiles[kc // 2][:, kc % 2, f * P:(f + 1) * P]

                xts = None
                for tt in range(N_TT):
                    s0 = (tt % QT_N) * 512
                    pss = [qkvp.tile([P, 512], F32, name="qkvps")
                           for _ in range(6)]
                    for kc in range(KC):
                        if tt == 0 and kc % 2 == 0 and kc // 2 + 4 < 16:
                            load_group(kc // 2 + 4)
                        xt = (xtiles0[kc // 2][:, kc % 2, :] if tt == 0
                              else xts[kc // 8][:, kc % 8, :])
                        for f in range(6):
                            nc.tensor.matmul(
                                pss[f][:], w_of(kc, f),
                                xt, start=(kc == 0), stop=(kc == KC - 1))
                    if tt + 1 < N_TT:
                        xts = load_x(tt + 1)
                    for f in range(6):
                        raw = stg.tile([P, 512], BF, name="raw")
                        nc.scalar.copy(raw[:], pss[f][:])
                        if f < 5:
                            # rope pair-swap via SBUF->SBUF DMA instead of a
                            # PE matmul (saves 40 matmuls + 40 ACT copies)
                            sw_sb = stg.tile([P, 512], BF, name="sw_sb")
                            e = nc.sync if f % 2 else nc.scalar
                            e.dma_start(sw_sb[1:128:2, :], raw[0:127:2, :])
                            e.dma_start(sw_sb[0:127:2, :], raw[1:128:2, :])
                            r1 = stg.tile([P, 512], BF, name="r1")
                            nc.vector.tensor_mul(r1[:], raw[:],
                                                 ropeA_sb[:, s0:s0 + 512])
                            r2 = stg.tile([P, 512], BF, name="r2")
                            nc.vector.tensor_mul(r2[:], sw_sb[:],
                                                 ropeB_sb[:, s0:s0 + 512])
                            dst = (qT[f] if f < HPC else kT)
                            nc.vector.tensor_add(
                                dst[:, tt * 512:(tt + 1) * 512], r1[:], r2[:])
                        else:
                            for j in range(4):
                                tp = miscp.tile([P, 512], BF, name="miscps")
                                nc.tensor.transpose(
                                    tp[:, :P], raw[:, j * P:(j + 1) * P],
                                    ident_sb[:])
                                nc.scalar.copy(vv[tt * 4 + j][:],
                                               tp[:, :P])

            # yfm allocated only after ph1 releases its SBUF
            late = top.enter_context(tc.tile_pool(name="late", bufs=1))
            yfm = [late.tile([P, NCORES, 512], BF, name=f"yfm{m}")
                   for m in range(HPC)]

            # wo prefetch pool: DMAs ride gpsimd/sync in ph2, gpsimd/scalar
            # in ph3 so the scalar queue stays free for attention exps
            wop = top.enter_context(tc.tile_pool(name="wop", bufs=2))
            wo_tiles = {}

            def wo_prefetch(ot, engs):
                t = wop.tile([P, KC, 512], BF, name="wo_sb")
                for g in range(8):
                    engs[g % len(engs)].dma_start(
                        t[:, g * 4:(g + 1) * 4, :],
                        woT_r[:, g * 4:(g + 1) * 4,
                              ot * 512:(ot + 1) * 512])
                wo_tiles[ot] = t

            wo_prefetch(0, [nc.gpsimd, nc.sync])
            wo_prefetch(1, [nc.gpsimd, nc.sync])

            # ---------------- phase 2: attention + A2A -------------------
            with ExitStack() as ph2, nc.named_scope("ph2_attn"):
                sp = ph2.enter_context(
                    tc.tile_pool(name="sp", bufs=2, space="PSUM"))
                yp = ph2.enter_context(
                    tc.tile_pool(name="yp", bufs=2, space="PSUM"))
                dp = ph2.enter_context(
                    tc.tile_pool(name="dp", bufs=2, space="PSUM"))
                ep = ph2.enter_context(tc.tile_pool(name="ep", bufs=6))
                esp = ph2.enter_context(tc.tile_pool(name="esp", bufs=5))
                ys = ph2.enter_context(tc.tile_pool(name="ys", bufs=5))
                rp = ph2.enter_context(tc.tile_pool(name="rp", bufs=3))

                def emit_norm(h, b, qt, yu_sb, b_sb):
                    y_sb = ys.tile([P, 512], BF, name="y_sb")
                    nc.vector.tensor_mul(y_sb[:], yu_sb[:], b_sb[:])
                    r = b * QT_N + qt
                    nc.sync.dma_start(
                        a2a_in[h][r * P:(r + 1) * P, :], y_sb[:])

                a2ao_r = [a2a_out[m][:].rearrange("(cc p) c -> p cc c", p=P)
                          for m in range(HPC)]
                yf_ready = []

                def flush_yf(before):
                    # flush only heads whose A2A was issued >=2 heads ago:
                    # a fresher flush still waits on the collective at the
                    # HEAD of the sync queue and blocks the emit_norm DMA
                    # stream behind it (vector then starves on y_sb WARs,
                    # and the PE pipeline stalls ~13us per head)
                    while yf_ready and yf_ready[0] < before:
                        hh = yf_ready.pop(0)
                        nc.sync.dma_start(yfm[hh][:, 0:4, :],
                                          a2ao_r[hh][:, 0:4, :])
                        nc.sync.dma_start(yfm[hh][:, 4:8, :],
                                          a2ao_r[hh][:, 4:8, :])

                def off_of(qt, kt):
                    c = cls[qt][kt]
                    return c[1] if isinstance(c, tuple) else 0

                # single flattened (h, b, qt) pipeline: heads are NOT
                # serialized, so the PE never drains at a head boundary
                # (the per-head tail used to idle the PE ~2.5us and HAM
                # re-throttled it to 1.2 GHz for another 3.4us)
                pending = []
                inflight = []   # (e_t, pair, ctx) awaiting AV + denom
                emitted = [0] * HPC
                deferred_quads = []

                def emit_norm(h, b, qt, yu_sb, b_sb):
                    y_sb2 = ys.tile([P, 512], BF, name="y_sb")
                    nc.vector.tensor_mul(y_sb2[:], yu_sb[:], b_sb[:])
                    r = b * QT_N + qt
                    nc.sync.dma_start(
                        a2a_in[h][r * P:(r + 1) * P, :], y_sb2[:])
                    emitted[h] += 1
                    if emitted[h] == B * QT_N:
                        nc.gpsimd.collective_compute(
                            "AllToAll", mybir.AluOpType.bypass,
                            replica_groups=rg,
                            ins=[a2a_in[h].opt()], outs=[a2a_out[h].opt()])
                        yf_ready.append(h)
                        flush_yf(h - 1)

                def finish_qt(ctx):
                    yu_sb = ys.tile([P, 512], F32, name="yu_sb")
                    nc.vector.tensor_copy(yu_sb[:], ctx['ps_y'][:])
                    rec = rp.tile([1, 512], F32, name="rec")
                    nc.vector.reciprocal_approx_fast(rec[:],
                                                     ctx['ps_d'][:])
                    b_sb = ys.tile([P, 512], F32, name="b_sb")
                    nc.gpsimd.partition_broadcast(b_sb[:], rec[:])
                    pending.append((ctx['h'], ctx['b'], ctx['qt'],
                                    yu_sb, b_sb))
                    if len(pending) > 3:
                        emit_norm(*pending.pop(0))

                def quad_mm(ctx, ap):
                    st = (ctx['quads_done'] == 0)
                    sf = (ctx['quads_done'] == ctx['total_quads'] - 1)
                    nc.tensor.matmul(
                        ctx['ps_d'][:, :], ones_sb[:], ap,
                        start=st, stop=sf, skip_group_check=True)
                    ctx['quads_done'] += 1

                def flush_quads():
                    while deferred_quads:
                        quad_mm(*deferred_quads.pop(0))

                def process_one():
                    flush_quads()
                    e_t, prr, ctx = inflight.pop(0)
                    b, qt = ctx['b'], ctx['qt']
                    for j, kt in enumerate(prr):
                        off = off_of(qt, kt)
                        st = (ctx['av_done'] == 0)
                        sf = (ctx['av_done'] == ctx['n_av'] - 1)
                        nc.tensor.matmul(
                            ctx['ps_y'][:, off:512], vv[b * KT_N + kt][:],
                            e_t[:, j * 512 + off:(j + 1) * 512],
                            start=st, stop=sf, skip_group_check=True)
                        ctx['av_done'] += 1
                    if len(prr) == 2:
                        s = esp.tile([P, 512], BF, name="es")
                        nc.vector.tensor_add(
                            s[:], e_t[:, 0:512], e_t[:, 512:1024])
                        ctx['quad'].append(s[:])
                    else:
                        ctx['quad'].append(e_t[:, 0:512])
                    if len(ctx['quad']) == 4 or (
                            ctx['av_done'] == ctx['n_av']
                            and ctx['quad']):
                        # DVE tree-reduce up to 4 pair-sums (8 kt) to
                        # one tile per ones-matmul: halves PE streaming
                        # spent on softmax denominators
                        q = ctx['quad']
                        while len(q) > 1:
                            nxt = []
                            for i2 in range(0, len(q) - 1, 2):
                                q2 = esp.tile([P, 512], BF, name="es2")
                                nc.vector.tensor_add(
                                    q2[:], q[i2], q[i2 + 1])
                                nxt.append(q2[:])
                            if len(q) % 2:
                                nxt.append(q[-1])
                            q = nxt
                        deferred_quads.append((ctx, q[0]))
                        ctx['quad'] = []
                    if ctx['av_done'] == ctx['n_av']:
                        flush_quads()
                        finish_qt(ctx)

                for h in range(HPC):
                    for b in range(B):
                        for qt in range(QT_N):
                            kts = [kt for kt in range(KT_N)
                                   if cls[qt][kt] != 'skip']
                            diag = [kt for kt in kts
                                    if isinstance(cls[qt][kt], tuple)]
                            if len(diag) == 4 and len(kts) % 2 == 0:
                                rest = [kt for kt in kts
                                        if kt not in diag]
                                dso = sorted(
                                    diag, key=lambda kt: -cls[qt][kt][1])
                                # (hi, hi2) pair first for max exp trim;
                                # (lo0, lo1) pair keeps first-AV full width
                                kts = rest + [dso[0], dso[1],
                                              dso[3], dso[2]]
                            pairs = [kts[i:i + 2]
                                     for i in range(0, len(kts), 2)]
                            ctx = dict(
                                h=h, b=b, qt=qt, n_av=len(kts), av_done=0,
                                quad=[], quads_done=0,
                                total_quads=(len(pairs) + 3) // 4,
                                ps_y=yp.tile([P, 512], F32, name="psy"),
                                ps_d=dp.tile([1, 512], F32, name="psd"))
                            for pr in pairs:
                                ps_s = sp.tile([P, 1024], F32, name="pss")
                                e_t = ep.tile([P, 1024], BF, name="e_t")
                                for j, kt in enumerate(pr):
                                    off = off_of(qt, kt)
                                    w = 512 - off
                                    q0 = b * S + qt * 512 + off
                                    nc.tensor.matmul(
                                        ps_s[:, j * 512 + off:
                                             (j + 1) * 512],
                                        kT[:, b * S + kt * P:
                                           b * S + (kt + 1) * P],
                                        qT[h][:, bass.ds(q0, w)],
                                        start=True, stop=True,
                                        skip_group_check=True)
                                # flat 2D activation over the whole pair;
                                # skip the leading all-masked region of the
                                # first kt (memset covers it below)
                                t0 = off_of(qt, pr[0])
                                nc.scalar.activation(
                                    e_t[:, t0:len(pr) * 512],
                                    ps_s[:, t0:len(pr) * 512],
                                    EXP, scale=SCALE)
                                tri_at = []
                                for j, kt in enumerate(pr):
                                    c = cls[qt][kt]
                                    off = off_of(qt, kt)
                                    j0 = j * 512
                                    if isinstance(c, tuple):
                                        if off:
                                            nc.vector.memset(
                                                e_t[:, j0:j0 + off], 0)
                                        tri_at.append(j0 + off)
                                    elif c != 'free':
                                        nc.vector.tensor_mul(
                                            e_t[:, j0:j0 + 512],
                                            e_t[:, j0:j0 + 512],
                                            mask_sb[c][:])
                                if len(tri_at) == 2:
                                    # both diag tri-blocks in one DVE op:
                                    # all offs are 128-aligned, so view the
                                    # pair tile as [P, 8, 128] and stride-
                                    # slice the two blocks; tri broadcasts
                                    # along the block axis
                                    bi0, bi1 = (tri_at[0] // P,
                                                tri_at[1] // P)
                                    ev = e_t[:, :].rearrange(
                                        "p (n f) -> p n f", f=P)
                                    e2 = ev[:, bi0:bi1 + 1:bi1 - bi0, :]
                                    t2 = tri_sb[:, :].rearrange(
                                        "p (n f) -> p n f", n=1)\
                                        .to_broadcast((P, 2, P))
                                    nc.vector.tensor_mul(e2, e2, t2)
                                elif tri_at:
                                    nc.vector.tensor_mul(
                                        e_t[:, tri_at[0]:tri_at[0] + P],
                                        e_t[:, tri_at[0]:tri_at[0] + P],
                                        tri_sb[:])
                                inflight.append((e_t, pr, ctx))
                                if len(inflight) > 2:
                                    process_one()
                while inflight:
                    process_one()
                for pn in pending:
                    emit_norm(*pn)
                flush_yf(HPC)

            # ---------------- phase 3: out projection --------------------
            with ExitStack() as ph3, nc.named_scope("ph3_outp"):
                opp = ph3.enter_context(
                    tc.tile_pool(name="opp", bufs=8, space="PSUM"))
                osb = ph3.enter_context(tc.tile_pool(name="osb", bufs=3))
                PH3_ENGS = [nc.gpsimd, nc.scalar, nc.sync]

                def yf_lhs(dc, ts):
                    m, cc = dc // NCORES, dc % NCORES
                    return yfm[m][:, cc, ts * P:(ts + 1) * P]

                def store(ot, ts, pso):
                    ob = osb.tile([P, 512], F32, name="ob")
                    nc.scalar.copy(ob[:], pso[:])
                    nc.sync.dma_start(
                        out_d[ts * P:(ts + 1) * P,
                              ot * 512:(ot + 1) * 512], ob[:])

                # prologue: all 8 PSUM banks accumulate heads 0-2
                # (dc 0-23) of ot0/ot1 before ANY head-3 matmul runs:
                # the A2As serialize on the CC engine (~26-40us each) and
                # A2A(3) completes ~45us after attention ends; this defers
                # the yfm[3] dependency ~50us of matmuls deep
                psos = {}
                for ot in range(2):
                    for ts in range(4):
                        pso = opp.tile([P, 512], F32, name="pso")
                        for dc in range(24):
                            nc.tensor.matmul(
                                pso[:], yf_lhs(dc, ts),
                                wo_tiles[ot][:, dc, :],
                                start=(dc == 0), stop=False,
                                skip_group_check=True)
                        psos[(ot, ts)] = pso
                for ot in range(2):
                    for ts in range(4):
                        pso = psos.pop((ot, ts))
                        for dc in range(24, KC):
                            nc.tensor.matmul(
                                pso[:], yf_lhs(dc, ts),
                                wo_tiles[ot][:, dc, :],
                                start=False, stop=(dc == KC - 1),
                                skip_group_check=True)
                        store(ot, ts, pso)
                    wo_tiles.pop(ot)
                    wo_prefetch(2 + ot, PH3_ENGS)
                for ot in range(2, 8):
                    wo_sb = wo_tiles.pop(ot)
                    if ot + 2 < 8:
                        wo_prefetch(ot + 2, PH3_ENGS)
                    for ts in range(4):
                        pso = opp.tile([P, 512], F32, name="pso")
                        for dc in range(KC):
                            nc.tensor.matmul(
                                pso[:], yf_lhs(dc, ts), wo_sb[:, dc, :],
                                start=(dc == 0), stop=(dc == KC - 1))
                        store(ot, ts, pso)

    nc.compile()
    return nc


# ---------------------------------------------------------------------------
# public entry
# ---------------------------------------------------------------------------

_CACHE = {}


def _execute(x, freqs_cis, mask_cache, input_pos, wqkv, wo,
             trace=False, debug=False):
    _install_profile_shim()
    prep = _prep(x, freqs_cis, mask_cache, wqkv, wo)
    cls = prep['cls']
    n_masks = 0 if prep['masks'] is None else prep['masks'].shape[0] // P
    key = (str(cls), n_masks, debug)
    if key not in _CACHE:
        _CACHE[key] = _build(cls, n_masks, debug=debug)
    nc = _CACHE[key]

    in_maps = []
    for c in range(NCORES):
        m = dict(xT=prep['xT'], wT=prep['wTs'][c], woT=prep['woT'],
                 ropeA=prep['ropeA'], ropeB=prep['ropeB'],
                 pswap=prep['pswap'], ident=prep['ident'],
                 ones=prep['ones'], tri=prep['tri'])
        if n_masks:
            m['masks'] = prep['masks']
        in_maps.append(m)

    res = run_bass_kernel_spmd(nc, in_maps, core_ids=list(range(NCORES)),
                               trace=trace,
                               trace_cores=list(range(NCORES)) if trace
                               else None)
    out = np.zeros((B, S, DIM), dtype=np.float32)
    for c in range(NCORES):
        b, j = c // QT_N, c % QT_N
        out[b, j * 512:(j + 1) * 512] = res.results[c]['out']
    return out, res


def kernel(x, freqs_cis, mask_cache, input_pos, wqkv, wo):
    out, _ = _execute(x, freqs_cis, mask_cache, input_pos, wqkv, wo)
    return out


# ---------------------------------------------------------------------------
# numpy simulation of the exact device pipeline (for validation)
# ---------------------------------------------------------------------------

def _simulate(x, freqs_cis, mask_cache, wqkv, wo, use_bf16=True):
    """Mirror the device computation in numpy.  Returns (out, debug_dict)."""
    def q_(a):  # quantize
        return a.astype(bf16).astype(np.float32) if use_bf16 else a

    prep = _prep(x, freqs_cis, mask_cache, wqkv, wo)
    cls = prep['cls']
    xT = prep['xT'].astype(np.float32)
    ropeA = np.concatenate([prep['ropeA'].astype(np.float32)] * B, axis=1)
    ropeB = np.concatenate([prep['ropeB'].astype(np.float32)] * B, axis=1)
    mask = np.asarray(mask_cache)[0, 0]

    dbg = {c: {} for c in range(NCORES)}
    a2a_ins = {m: [] for m in range(HPC)}  # m -> [core][8*128, 512]
    for c in range(NCORES):
        wT = prep['wTs'][c].astype(np.float32)
        qkvT = q_(wT.T @ xT)       # [768, TOK]  (psum f32, evict to bf16)
        sw = np.empty_like(qkvT[:5 * P])
        for f in range(5):
            blk = qkvT[f * P:(f + 1) * P]
            sw[f * P:(f + 1) * P] = q_(blk[[i ^ 1 for i in range(P)], :])
        roped = np.empty_like(qkvT[:5 * P])
        for f in range(5):
            blk = qkvT[f * P:(f + 1) * P]
            r1 = q_(blk * ropeA)
            r2 = q_(sw[f * P:(f + 1) * P] * ropeB)
            roped[f * P:(f + 1) * P] = q_(r1 + r2)
        qTs = [roped[h * P:(h + 1) * P] for h in range(HPC)]
        kTc = roped[4 * P:5 * P]
        vT = qkvT[5 * P:6 * P]     # [128 d, TOK], not roped
        for h in range(HPC):
            a2a_c = np.zeros((NCORES * P, 512), dtype=np.float32)
            for b in range(B):
                kTb = kTc[:, b * S:(b + 1) * S]
                vTb = vT[:, b * S:(b + 1) * S]
                qTb = qTs[h][:, b * S:(b + 1) * S]
                sT = kTb.T @ qTb               # [Sk, Sq] psum f32
                e = q_(np.exp(sT * SCALE))     # ACT exp -> bf16
                emask = e * mask.T             # mask multiply (exact 0/1)
                for qt in range(QT_N):
                    for kt in range(KT_N):
                        if cls[qt][kt] == 'skip':
                            emask[kt * P:(kt + 1) * P,
                                  qt * 512:(qt + 1) * 512] = 0
                D = emask.sum(axis=0)          # psum f32
                rec = 1.0 / D
                yTu = vTb @ emask
                y = q_(yTu * rec[None, :])
                for qt in range(QT_N):
                    r = b * QT_N + qt
                    a2a_c[r * P:(r + 1) * P] = y[:, qt * 512:(qt + 1) * 512]
            a2a_ins[h].append(a2a_c)

    out_full = np.zeros((B, S, DIM), dtype=np.float32)
    woT = prep['woT'].astype(np.float32)
    for c in range(NCORES):
        yfull = np.zeros((DIM, 512), dtype=np.float32)
        for m in range(HPC):
            for j in range(NCORES):
                dbi = m * NCORES + j
                yfull[dbi * P:(dbi + 1) * P] = \
                    a2a_ins[m][j][c * P:(c + 1) * P]
        o = yfull.T @ woT          # [512 tok, DIM] psum f32
        b, jj = c // QT_N, c % QT_N
        out_full[b, jj * 512:(jj + 1) * 512] = o
    return out_full, dbg



# revision 29
# speedup vs baseline: 1.0139x; 1.0139x over previous
"""Distributed Trainium2 kernel for nn_Attention_37958920962105.

GQA attention layer (DIM=4096, 32 q heads, 8 kv heads, head_dim=128,
B=2, S=2048) sharded tensor-parallel over GQA groups across 8 cores:
core c owns q heads 4c..4c+3 and kv head c.  Per core:
  1. QKV projection (transposed layouts) + RoPE + v transpose
  2. attention: scores (2 kt per PSUM pair-tile) -> one mega-exp ACT per
     pair -> AV matmuls; softmax denominators via DVE quad-tree sums +
     one ones-matmul per 4 kt tiles (cuts PE streaming 4x)
  3. AllToAll (one per local head) to token-shard y
  4. out projection on this core's 512-token chunk
Output chunks are reassembled on the host.
"""

import math
import sys
import types
from contextlib import ExitStack

import numpy as np
import ml_dtypes

import concourse.bass as bass
import concourse.mybir as mybir
import concourse.tile as tile
from concourse import bacc
from concourse.bass_utils import run_bass_kernel_spmd

BF = mybir.dt.bfloat16
F32 = mybir.dt.float32
bf16 = ml_dtypes.bfloat16

P = 128
DIM = 4096
N_HEAD = 32
N_KV = 8
HD = 128
B = 2
S = 2048
TOK = B * S          # 4096
NCORES = 8
HPC = N_HEAD // N_KV  # 4 q heads per core
FQKV = (HPC + 2) * HD  # 768 qkv rows per core
KC = DIM // P        # 32 contraction chunks
N_TT = TOK // 512    # 8 token tiles of 512
QT_N = S // 512      # 4 q tiles per batch
KT_N = S // P        # 16 k tiles per batch
SCALE = 1.0 / math.sqrt(HD)


def _install_profile_shim():
    if 'antenv.axon_hooks' in sys.modules:
        return
    try:
        from trn_agent_boot.trn_boot import _ntff_profile_via_ctypes
        hook = _ntff_profile_via_ctypes('/opt/axon/libaxon_pjrt.so')
    except Exception:
        hook = None
    mod = types.ModuleType('antenv.axon_hooks')
    mod._hook = hook
    mod.get_axon_ntff_profile_hook = lambda: mod._hook
    mod.set_axon_ntff_profile_hook = lambda h: setattr(mod, '_hook', h)
    sys.modules['antenv.axon_hooks'] = mod
    try:
        import antenv
        antenv.axon_hooks = mod
    except ImportError:
        pass


# ---------------------------------------------------------------------------
# host-side prep
# ---------------------------------------------------------------------------

def _classify_mask(mask):
    """mask: [S(q), S(k)] bool.  Returns (cls, mask_tiles) where
    cls[qt][kt] in {'skip', 'free', ('diag', off), int mask-tile-index};
    mask tiles are transposed [128 k, 512 q] bf16."""
    cls = [[None] * KT_N for _ in range(QT_N)]
    tiles = []
    seen = {}
    qi = np.arange(512)[:, None]
    ki = np.arange(P)[None, :]
    for qt in range(QT_N):
        for kt in range(KT_N):
            blk = mask[qt * 512:(qt + 1) * 512, kt * P:(kt + 1) * P]
            if not blk.any():
                cls[qt][kt] = 'skip'
                continue
            if blk.all():
                cls[qt][kt] = 'free'
                continue
            off = kt * P - qt * 512
            if 0 <= off < 512 and np.array_equal(blk, (off + ki) <= qi):
                cls[qt][kt] = ('diag', off)
                continue
            key = blk.tobytes()
            if key not in seen:
                seen[key] = len(tiles)
                tiles.append(np.ascontiguousarray(blk.T).astype(bf16))
            cls[qt][kt] = seen[key]
    return cls, tiles


def _prep(x, freqs_cis, mask_cache, wqkv, wo):
    x = np.asarray(x, dtype=np.float32)
    freqs_cis = np.asarray(freqs_cis, dtype=np.float32)
    wqkv = np.asarray(wqkv, dtype=np.float32)
    wo = np.asarray(wo, dtype=np.float32)
    mask = np.asarray(mask_cache)[0, 0]

    xT = np.ascontiguousarray(x.reshape(TOK, DIM).T).astype(bf16)

    wTs = []
    for c in range(NCORES):
        w_c = np.concatenate([
            wqkv[HPC * HD * c: HPC * HD * (c + 1)],          # 4 q heads
            wqkv[N_HEAD * HD + HD * c: N_HEAD * HD + HD * (c + 1)],   # k head
            wqkv[(N_HEAD + N_KV) * HD + HD * c:
                 (N_HEAD + N_KV) * HD + HD * (c + 1)],       # v head
        ], axis=0)                                           # [768, DIM]
        wTs.append(np.ascontiguousarray(w_c.T).astype(bf16))  # [DIM, 768]

    # wo permuted so row-block dbi = m*8 + cc holds global head 4*cc + m
    woT = np.ascontiguousarray(wo.T)                 # [d, o]
    woT_h = woT.reshape(N_HEAD, HD, DIM)
    perm = [4 * (dbi % NCORES) + dbi // NCORES for dbi in range(N_HEAD)]
    woT_perm = np.ascontiguousarray(woT_h[perm].reshape(DIM, DIM)).astype(bf16)

    f0 = freqs_cis[:, :, 0].T                        # [64, S]
    f1 = freqs_cis[:, :, 1].T
    ropeA = np.repeat(f0, 2, axis=0).astype(bf16)    # [128, S]
    ropeB = np.empty((HD, S), dtype=np.float32)
    ropeB[0::2] = -f1
    ropeB[1::2] = f1
    ropeB = ropeB.astype(bf16)

    pswap = np.zeros((P, P), dtype=bf16)
    for i in range(P):
        pswap[i, i ^ 1] = 1
    ident = np.eye(P, dtype=bf16)
    ones_col = np.ones((P, 1), dtype=bf16)

    tri = (np.arange(P)[:, None] <= np.arange(P)[None, :]).astype(bf16)
    cls, mask_tiles = _classify_mask(mask)
    masks = (np.concatenate([t for t in mask_tiles], axis=0)
             if mask_tiles else None)                # [n*128, 512] bf16

    return dict(xT=xT, wTs=wTs, woT=woT_perm, ropeA=ropeA, ropeB=ropeB,
                pswap=pswap, ident=ident, ones=ones_col,
                tri=tri, cls=cls, masks=masks)


# ---------------------------------------------------------------------------
# device kernel builder
# ---------------------------------------------------------------------------

def _build(cls, n_masks, debug=False):
    nc = bacc.Bacc("TRN2", target_bir_lowering=False, debug=False,
                   num_devices=NCORES)
    xT_d = nc.dram_tensor("xT", [DIM, TOK], BF, kind="ExternalInput")
    wT_d = nc.dram_tensor("wT", [DIM, FQKV], BF, kind="ExternalInput")
    woT_d = nc.dram_tensor("woT", [DIM, DIM], BF, kind="ExternalInput")
    ropeA_d = nc.dram_tensor("ropeA", [P, S], BF, kind="ExternalInput")
    ropeB_d = nc.dram_tensor("ropeB", [P, S], BF, kind="ExternalInput")
    pswap_d = nc.dram_tensor("pswap", [P, P], BF, kind="ExternalInput")
    ident_d = nc.dram_tensor("ident", [P, P], BF, kind="ExternalInput")
    ones_d = nc.dram_tensor("ones", [P, 1], BF, kind="ExternalInput")
    tri_d = nc.dram_tensor("tri", [P, P], BF, kind="ExternalInput")
    masks_d = (nc.dram_tensor("masks", [n_masks * P, 512], BF,
                              kind="ExternalInput") if n_masks else None)
    out_d = nc.dram_tensor("out", [512, DIM], F32, kind="ExternalOutput")

    EXP = mybir.ActivationFunctionType.Exp
    rg = [list(range(NCORES))]

    wT_r = wT_d[:, :].rearrange("(k p) f -> p k f", p=P)      # [P, KC, FQKV]
    xT_r = xT_d[:, :].rearrange("(k p) t -> p k t", p=P)      # [P, KC, TOK]
    woT_r = woT_d[:, :].rearrange("(k p) o -> p k o", p=P)    # [P, KC, DIM]

    with tile.TileContext(nc) as tc:
        with ExitStack() as top:
            const = top.enter_context(tc.tile_pool(name="const", bufs=1))
            acts = top.enter_context(tc.tile_pool(name="acts", bufs=1))
            dramp = top.enter_context(tc.tile_pool(name="dramp", bufs=1,
                                                   space="DRAM"))

            qT = [acts.tile([P, TOK], BF, name=f"qT{h}") for h in range(HPC)]
            kT = acts.tile([P, TOK], BF, name="kT")
            vv = [acts.tile([P, P], BF, name=f"v{i}") for i in range(TOK // P)]

            a2a_in = [dramp.tile([NCORES * P, 512], BF, name=f"a2ai{m}")
                      for m in range(HPC)]
            a2a_out = [dramp.tile([NCORES * P, 512], BF, name=f"a2ao{m}")
                       for m in range(HPC)]
            # tiny dummy A2A issued during ph1: absorbs the ~14us
            # first-collective setup so the four real (serialized) A2As
            # all complete earlier relative to ph3's yfm reads
            warm_i = dramp.tile([NCORES, 512], BF, name="warm_i")
            warm_o = dramp.tile([NCORES, 512], BF, name="warm_o")

            # ---------------- PE warmup during initial DMA wait ----------
            # junk lives in the never-freed const pool: if it sat in a
            # scratch pool, the first wqkv DMA would land on its address
            # and wait (WAR) for every warmup matmul before loading
            with ExitStack() as wu, nc.named_scope("warmup"):
                wups = wu.enter_context(tc.tile_pool(name="wups", bufs=1,
                                                     space="PSUM"))
                junk = const.tile([P, 256], BF, name="junk")
                nc.vector.memset(junk[:], 0)
                wps = wups.tile([P, 256], F32, name="wps")
                for _ in range(8):
                    nc.tensor.matmul(wps[:], junk[:, :P], junk[:],
                                     start=True, stop=True,
                                     skip_group_check=True)

            # ---------------- phase 1: QKV + rope + v transpose ----------
            with ExitStack() as ph1, nc.named_scope("ph1_qkv"):
                wp = ph1.enter_context(tc.tile_pool(name="wp", bufs=1))
                xp = ph1.enter_context(tc.tile_pool(name="xp", bufs=6))
                xp0 = ph1.enter_context(tc.tile_pool(name="xp0", bufs=16))

                def load_x(tt):
                    tiles = []
                    for g in range(4):
                        t = xp.tile([P, 8, 512], BF, name="xt8")
                        (nc.scalar if g % 2 else nc.sync).dma_start(
                            t[:], xT_r[:, g * 8:(g + 1) * 8,
                                       tt * 512:(tt + 1) * 512])
                        tiles.append(t)
                    return tiles

                wtiles = []
                xtiles0 = []

                def load_group(g):
                    # per-kc granularity in consumption order: startup is
                    # HBM-bound with all 8 cores racing, so the first MM's
                    # deps must head their queues.  g0 w on sync (HWDGE,
                    # fastest first-byte) with the f0 32KB slice split out,
                    # x0 + g1 on scalar, g2/g3 ahead of consts on gpsimd
                    if g == 0:
                        ew, ex = nc.sync, nc.scalar
                    elif g == 1:
                        ew = ex = nc.scalar
                    elif g < 4:
                        ew = ex = nc.gpsimd
                    else:
                        ew, ex = ((nc.sync, nc.scalar) if g % 2 else
                                  (nc.scalar, nc.sync))
                    w2 = wp.tile([P, 2, FQKV], BF, name=f"w2_{g}")
                    x2 = xp0.tile([P, 2, 512], BF, name="x2")
                    for j in range(2):
                        if g == 0 and j == 0:
                            ew.dma_start(w2[:, j, 0:P],
                                         wT_r[:, g * 2 + j, 0:P])
                            ew.dma_start(w2[:, j, P:],
                                         wT_r[:, g * 2 + j, P:])
                        else:
                            ew.dma_start(w2[:, j, :],
                                         wT_r[:, g * 2 + j, :])
                        ex.dma_start(x2[:, j, :],
                                     xT_r[:, g * 2 + j, 0:512])
                    wtiles.append(w2)
                    xtiles0.append(x2)

                qkvp = ph1.enter_context(
                    tc.tile_pool(name="qkvp", bufs=6, space="PSUM"))
                miscp = ph1.enter_context(
                    tc.tile_pool(name="miscp", bufs=2, space="PSUM"))
                stg = ph1.enter_context(tc.tile_pool(name="stg", bufs=5))

                for g in range(4):
                    load_group(g)

                nc.gpsimd.collective_compute(
                    "AllToAll", mybir.AluOpType.bypass,
                    replica_groups=rg,
                    ins=[warm_i.opt()], outs=[warm_o.opt()])

                ropeA_sb = const.tile([P, S], BF, name="ropeA_sb")
                nc.gpsimd.dma_start(ropeA_sb[:], ropeA_d[:])
                ropeB_sb = const.tile([P, S], BF, name="ropeB_sb")
                nc.gpsimd.dma_start(ropeB_sb[:], ropeB_d[:])
                pswap_sb = const.tile([P, P], BF, name="pswap_sb")
                nc.gpsimd.dma_start(pswap_sb[:], pswap_d[:])
                ident_sb = const.tile([P, P], BF, name="ident_sb")
                nc.gpsimd.dma_start(ident_sb[:], ident_d[:])
                ones_sb = const.tile([P, 1], BF, name="ones_sb")
                nc.gpsimd.dma_start(ones_sb[:], ones_d[:])
                tri_sb = const.tile([P, P], BF, name="tri_sb")
                nc.gpsimd.dma_start(tri_sb[:], tri_d[:])
                mask_sb = []
                for i in range(n_masks):
                    m = const.tile([P, 512], BF, name=f"mask{i}")
                    nc.gpsimd.dma_start(m[:], masks_d[i * P:(i + 1) * P, :])
                    mask_sb.append(m)

                def w_of(kc, f):
                    return wtiles[kc // 2][:, kc % 2, f * P:(f + 1) * P]

                xts = None
                for tt in range(N_TT):
                    s0 = (tt % QT_N) * 512
                    pss = [qkvp.tile([P, 512], F32, name="qkvps")
                           for _ in range(6)]
                    for kc in range(KC):
                        if tt == 0 and kc % 2 == 0 and kc // 2 + 4 < 16:
                            load_group(kc // 2 + 4)
                        xt = (xtiles0[kc // 2][:, kc % 2, :] if tt == 0
                              else xts[kc // 8][:, kc % 8, :])
                        for f in range(6):
                            nc.tensor.matmul(
                                pss[f][:], w_of(kc, f),
                                xt, start=(kc == 0), stop=(kc == KC - 1))
                    if tt + 1 < N_TT:
                        xts = load_x(tt + 1)
                    for f in range(6):
                        raw = stg.tile([P, 512], BF, name="raw")
                        nc.scalar.copy(raw[:], pss[f][:])
                        if f < 5:
                            # rope pair-swap via SBUF->SBUF DMA instead of a
                            # PE matmul (saves 40 matmuls + 40 ACT copies)
                            sw_sb = stg.tile([P, 512], BF, name="sw_sb")
                            e = nc.sync if f % 2 else nc.scalar
                            e.dma_start(sw_sb[1:128:2, :], raw[0:127:2, :])
                            e.dma_start(sw_sb[0:127:2, :], raw[1:128:2, :])
                            r1 = stg.tile([P, 512], BF, name="r1")
                            nc.vector.tensor_mul(r1[:], raw[:],
                                                 ropeA_sb[:, s0:s0 + 512])
                            r2 = stg.tile([P, 512], BF, name="r2")
                            nc.vector.tensor_mul(r2[:], sw_sb[:],
                                                 ropeB_sb[:, s0:s0 + 512])
                            dst = (qT[f] if f < HPC else kT)
                            nc.vector.tensor_add(
                                dst[:, tt * 512:(tt + 1) * 512], r1[:], r2[:])
                        else:
                            for j in range(4):
                                tp = miscp.tile([P, 512], BF, name="miscps")
                                nc.tensor.transpose(
                                    tp[:, :P], raw[:, j * P:(j + 1) * P],
                                    ident_sb[:])
                                nc.scalar.copy(vv[tt * 4 + j][:],
                                               tp[:, :P])

            # yfm allocated only after ph1 releases its SBUF
            late = top.enter_context(tc.tile_pool(name="late", bufs=1))
            yfm = [late.tile([P, NCORES, 512], BF, name=f"yfm{m}")
                   for m in range(HPC)]

            # wo prefetch pool: DMAs ride gpsimd/sync in ph2, gpsimd/scalar
            # in ph3 so the scalar queue stays free for attention exps
            wop = top.enter_context(tc.tile_pool(name="wop", bufs=2))
            wo_tiles = {}

            def wo_prefetch(ot, engs, pool=None):
                t = (pool or wop).tile([P, KC, 512], BF, name="wo_sb")
                for g in range(8):
                    engs[g % len(engs)].dma_start(
                        t[:, g * 4:(g + 1) * 4, :],
                        woT_r[:, g * 4:(g + 1) * 4,
                              ot * 512:(ot + 1) * 512])
                wo_tiles[ot] = t

            wo_prefetch(0, [nc.gpsimd, nc.sync])
            wo_prefetch(1, [nc.gpsimd, nc.sync])

            # ---------------- phase 2: attention + A2A -------------------
            with ExitStack() as ph2, nc.named_scope("ph2_attn"):
                sp = ph2.enter_context(
                    tc.tile_pool(name="sp", bufs=2, space="PSUM"))
                yp = ph2.enter_context(
                    tc.tile_pool(name="yp", bufs=2, space="PSUM"))
                dp = ph2.enter_context(
                    tc.tile_pool(name="dp", bufs=2, space="PSUM"))
                ep = ph2.enter_context(tc.tile_pool(name="ep", bufs=6))
                esp = ph2.enter_context(tc.tile_pool(name="esp", bufs=5))
                ys = ph2.enter_context(tc.tile_pool(name="ys", bufs=5))
                rp = ph2.enter_context(tc.tile_pool(name="rp", bufs=3))

                def emit_norm(h, b, qt, yu_sb, b_sb):
                    y_sb = ys.tile([P, 512], BF, name="y_sb")
                    nc.vector.tensor_mul(y_sb[:], yu_sb[:], b_sb[:])
                    r = b * QT_N + qt
                    nc.sync.dma_start(
                        a2a_in[h][r * P:(r + 1) * P, :], y_sb[:])

                a2ao_r = [a2a_out[m][:].rearrange("(cc p) c -> p cc c", p=P)
                          for m in range(HPC)]
                yf_ready = []

                def flush_yf(before):
                    # flush head h-1 when A2A(h) is issued: with the CC
                    # engine pre-warmed an A2A completes in ~16us, well
                    # under the ~53us head cadence, so the flush never
                    # waits at the head of the sync queue (which would
                    # block the emit_norm DMA stream behind it and stall
                    # the whole vector->tensor pipeline)
                    while yf_ready and yf_ready[0] < before:
                        hh = yf_ready.pop(0)
                        nc.sync.dma_start(yfm[hh][:, 0:4, :],
                                          a2ao_r[hh][:, 0:4, :])
                        nc.sync.dma_start(yfm[hh][:, 4:8, :],
                                          a2ao_r[hh][:, 4:8, :])

                def off_of(qt, kt):
                    c = cls[qt][kt]
                    return c[1] if isinstance(c, tuple) else 0

                # single flattened (h, b, qt) pipeline: heads are NOT
                # serialized, so the PE never drains at a head boundary
                # (the per-head tail used to idle the PE ~2.5us and HAM
                # re-throttled it to 1.2 GHz for another 3.4us)
                pending = []
                inflight = []   # (e_t, pair, ctx) awaiting AV + denom
                emitted = [0] * HPC
                deferred_quads = []

                def emit_norm(h, b, qt, yu_sb, b_sb):
                    y_sb2 = ys.tile([P, 512], BF, name="y_sb")
                    nc.vector.tensor_mul(y_sb2[:], yu_sb[:], b_sb[:])
                    r = b * QT_N + qt
                    nc.sync.dma_start(
                        a2a_in[h][r * P:(r + 1) * P, :], y_sb2[:])
                    emitted[h] += 1
                    if emitted[h] == B * QT_N:
                        nc.gpsimd.collective_compute(
                            "AllToAll", mybir.AluOpType.bypass,
                            replica_groups=rg,
                            ins=[a2a_in[h].opt()], outs=[a2a_out[h].opt()])
                        yf_ready.append(h)
                        flush_yf(h)

                def finish_qt(ctx):
                    yu_sb = ys.tile([P, 512], F32, name="yu_sb")
                    nc.vector.tensor_copy(yu_sb[:], ctx['ps_y'][:])
                    rec = rp.tile([1, 512], F32, name="rec")
                    nc.vector.reciprocal_approx_fast(rec[:],
                                                     ctx['ps_d'][:])
                    b_sb = ys.tile([P, 512], F32, name="b_sb")
                    nc.gpsimd.partition_broadcast(b_sb[:], rec[:])
                    pending.append((ctx['h'], ctx['b'], ctx['qt'],
                                    yu_sb, b_sb))
                    if len(pending) > 3:
                        emit_norm(*pending.pop(0))

                def quad_mm(ctx, ap):
                    st = (ctx['quads_done'] == 0)
                    sf = (ctx['quads_done'] == ctx['total_quads'] - 1)
                    nc.tensor.matmul(
                        ctx['ps_d'][:, :], ones_sb[:], ap,
                        start=st, stop=sf, skip_group_check=True)
                    ctx['quads_done'] += 1

                def flush_quads():
                    while deferred_quads:
                        quad_mm(*deferred_quads.pop(0))

                def process_one():
                    flush_quads()
                    e_t, prr, ctx = inflight.pop(0)
                    b, qt = ctx['b'], ctx['qt']
                    for j, kt in enumerate(prr):
                        off = off_of(qt, kt)
                        st = (ctx['av_done'] == 0)
                        sf = (ctx['av_done'] == ctx['n_av'] - 1)
                        nc.tensor.matmul(
                            ctx['ps_y'][:, off:512], vv[b * KT_N + kt][:],
                            e_t[:, j * 512 + off:(j + 1) * 512],
                            start=st, stop=sf, skip_group_check=True)
                        ctx['av_done'] += 1
                    if len(prr) == 2:
                        s = esp.tile([P, 512], BF, name="es")
                        nc.vector.tensor_add(
                            s[:], e_t[:, 0:512], e_t[:, 512:1024])
                        ctx['quad'].append(s[:])
                    else:
                        ctx['quad'].append(e_t[:, 0:512])
                    if len(ctx['quad']) == 4 or (
                            ctx['av_done'] == ctx['n_av']
                            and ctx['quad']):
                        # DVE tree-reduce up to 4 pair-sums (8 kt) to
                        # one tile per ones-matmul: halves PE streaming
                        # spent on softmax denominators
                        q = ctx['quad']
                        while len(q) > 1:
                            nxt = []
                            for i2 in range(0, len(q) - 1, 2):
                                q2 = esp.tile([P, 512], BF, name="es2")
                                nc.vector.tensor_add(
                                    q2[:], q[i2], q[i2 + 1])
                                nxt.append(q2[:])
                            if len(q) % 2:
                                nxt.append(q[-1])
                            q = nxt
                        deferred_quads.append((ctx, q[0]))
                        ctx['quad'] = []
                    if ctx['av_done'] == ctx['n_av']:
                        flush_quads()
                        finish_qt(ctx)

                for h in range(HPC):
                    for b in range(B):
                        for qt in range(QT_N):
                            kts = [kt for kt in range(KT_N)
                                   if cls[qt][kt] != 'skip']
                            diag = [kt for kt in kts
                                    if isinstance(cls[qt][kt], tuple)]
                            if len(diag) == 4 and len(kts) % 2 == 0:
                                rest = [kt for kt in kts
                                        if kt not in diag]
                                dso = sorted(
                                    diag, key=lambda kt: -cls[qt][kt][1])
                                # (hi, hi2) pair first for max exp trim;
                                # (lo0, lo1) pair keeps first-AV full width
                                kts = rest + [dso[0], dso[1],
                                              dso[3], dso[2]]
                            pairs = [kts[i:i + 2]
                                     for i in range(0, len(kts), 2)]
                            ctx = dict(
                                h=h, b=b, qt=qt, n_av=len(kts), av_done=0,
                                quad=[], quads_done=0,
                                total_quads=(len(pairs) + 3) // 4,
                                ps_y=yp.tile([P, 512], F32, name="psy"),
                                ps_d=dp.tile([1, 512], F32, name="psd"))
                            for pr in pairs:
                                ps_s = sp.tile([P, 1024], F32, name="pss")
                                e_t = ep.tile([P, 1024], BF, name="e_t")
                                for j, kt in enumerate(pr):
                                    off = off_of(qt, kt)
                                    w = 512 - off
                                    q0 = b * S + qt * 512 + off
                                    nc.tensor.matmul(
                                        ps_s[:, j * 512 + off:
                                             (j + 1) * 512],
                                        kT[:, b * S + kt * P:
                                           b * S + (kt + 1) * P],
                                        qT[h][:, bass.ds(q0, w)],
                                        start=True, stop=True,
                                        skip_group_check=True)
                                # flat 2D activation over the whole pair;
                                # skip the leading all-masked region of the
                                # first kt (memset covers it below)
                                t0 = off_of(qt, pr[0])
                                nc.scalar.activation(
                                    e_t[:, t0:len(pr) * 512],
                                    ps_s[:, t0:len(pr) * 512],
                                    EXP, scale=SCALE)
                                tri_at = []
                                for j, kt in enumerate(pr):
                                    c = cls[qt][kt]
                                    off = off_of(qt, kt)
                                    j0 = j * 512
                                    if isinstance(c, tuple):
                                        if off:
                                            nc.vector.memset(
                                                e_t[:, j0:j0 + off], 0)
                                        tri_at.append(j0 + off)
                                    elif c != 'free':
                                        nc.vector.tensor_mul(
                                            e_t[:, j0:j0 + 512],
                                            e_t[:, j0:j0 + 512],
                                            mask_sb[c][:])
                                if len(tri_at) == 2:
                                    # both diag tri-blocks in one DVE op:
                                    # all offs are 128-aligned, so view the
                                    # pair tile as [P, 8, 128] and stride-
                                    # slice the two blocks; tri broadcasts
                                    # along the block axis
                                    bi0, bi1 = (tri_at[0] // P,
                                                tri_at[1] // P)
                                    ev = e_t[:, :].rearrange(
                                        "p (n f) -> p n f", f=P)
                                    e2 = ev[:, bi0:bi1 + 1:bi1 - bi0, :]
                                    t2 = tri_sb[:, :].rearrange(
                                        "p (n f) -> p n f", n=1)\
                                        .to_broadcast((P, 2, P))
                                    nc.vector.tensor_mul(e2, e2, t2)
                                elif tri_at:
                                    nc.vector.tensor_mul(
                                        e_t[:, tri_at[0]:tri_at[0] + P],
                                        e_t[:, tri_at[0]:tri_at[0] + P],
                                        tri_sb[:])
                                inflight.append((e_t, pr, ctx))
                                if len(inflight) > 2:
                                    process_one()
                while inflight:
                    process_one()
                for pn in pending:
                    emit_norm(*pn)
                flush_yf(HPC)

            # ---------------- phase 3: out projection --------------------
            with ExitStack() as ph3, nc.named_scope("ph3_outp"):
                opp = ph3.enter_context(
                    tc.tile_pool(name="opp", bufs=8, space="PSUM"))
                osb = ph3.enter_context(tc.tile_pool(name="osb", bufs=3))
                wop2 = ph3.enter_context(tc.tile_pool(name="wop2", bufs=1))
                PH3_ENGS = [nc.gpsimd, nc.scalar, nc.sync]

                def yf_lhs(dc, ts):
                    m, cc = dc // NCORES, dc % NCORES
                    return yfm[m][:, cc, ts * P:(ts + 1) * P]

                def store(ot, ts, pso):
                    ob = osb.tile([P, 512], F32, name="ob")
                    nc.scalar.copy(ob[:], pso[:])
                    nc.sync.dma_start(
                        out_d[ts * P:(ts + 1) * P,
                              ot * 512:(ot + 1) * 512], ob[:])

                # prologue: all 8 PSUM banks accumulate ot0/ot1 in
                # head-bands (dc 0-15, then 16-23, then 24-31): the A2As
                # serialize on the CC engine (~16us each warm) finishing
                # ~10/25/40us after attention ends, so each yfm[m] is
                # touched only after enough earlier-band matmuls cover
                # its flush
                psos = {}
                for band_lo, band_hi in ((0, 16), (16, 24), (24, KC)):
                    for ot in range(2):
                        for ts in range(4):
                            if band_lo == 0:
                                psos[(ot, ts)] = opp.tile(
                                    [P, 512], F32, name="pso")
                            pso = psos[(ot, ts)]
                            for dc in range(band_lo, band_hi):
                                nc.tensor.matmul(
                                    pso[:], yf_lhs(dc, ts),
                                    wo_tiles[ot][:, dc, :],
                                    start=(dc == 0), stop=(dc == KC - 1),
                                    skip_group_check=True)
                            if band_hi == KC:
                                store(ot, ts, pso)
                    if band_lo == 0:
                        # third wo buffer lives in SBUF freed by ph2's
                        # pools, so this prefetch has no WAR on
                        # wo_tiles[0/1] and streams during the prologue
                        wo_prefetch(2, PH3_ENGS, pool=wop2)
                for ot in range(2):
                    wo_tiles.pop(ot)
                    if ot + 3 < 8:
                        wo_prefetch(ot + 3, PH3_ENGS)
                for ot in range(2, 8):
                    wo_sb = wo_tiles.pop(ot)
                    if ot + 3 < 8:
                        wo_prefetch(ot + 3, PH3_ENGS,
                                    pool=(wop2 if (ot + 3) % 3 == 2
                                          else wop))
                    for ts in range(4):
                        pso = opp.tile([P, 512], F32, name="pso")
                        for dc in range(KC):
                            nc.tensor.matmul(
                                pso[:], yf_lhs(dc, ts), wo_sb[:, dc, :],
                                start=(dc == 0), stop=(dc == KC - 1))
                        store(ot, ts, pso)

    nc.compile()
    return nc


# ---------------------------------------------------------------------------
# public entry
# ---------------------------------------------------------------------------

_CACHE = {}


def _execute(x, freqs_cis, mask_cache, input_pos, wqkv, wo,
             trace=False, debug=False):
    _install_profile_shim()
    prep = _prep(x, freqs_cis, mask_cache, wqkv, wo)
    cls = prep['cls']
    n_masks = 0 if prep['masks'] is None else prep['masks'].shape[0] // P
    key = (str(cls), n_masks, debug)
    if key not in _CACHE:
        _CACHE[key] = _build(cls, n_masks, debug=debug)
    nc = _CACHE[key]

    in_maps = []
    for c in range(NCORES):
        m = dict(xT=prep['xT'], wT=prep['wTs'][c], woT=prep['woT'],
                 ropeA=prep['ropeA'], ropeB=prep['ropeB'],
                 pswap=prep['pswap'], ident=prep['ident'],
                 ones=prep['ones'], tri=prep['tri'])
        if n_masks:
            m['masks'] = prep['masks']
        in_maps.append(m)

    res = run_bass_kernel_spmd(nc, in_maps, core_ids=list(range(NCORES)),
                               trace=trace,
                               trace_cores=list(range(NCORES)) if trace
                               else None)
    out = np.zeros((B, S, DIM), dtype=np.float32)
    for c in range(NCORES):
        b, j = c // QT_N, c % QT_N
        out[b, j * 512:(j + 1) * 512] = res.results[c]['out']
    return out, res


def kernel(x, freqs_cis, mask_cache, input_pos, wqkv, wo):
    out, _ = _execute(x, freqs_cis, mask_cache, input_pos, wqkv, wo)
    return out


# ---------------------------------------------------------------------------
# numpy simulation of the exact device pipeline (for validation)
# ---------------------------------------------------------------------------

def _simulate(x, freqs_cis, mask_cache, wqkv, wo, use_bf16=True):
    """Mirror the device computation in numpy.  Returns (out, debug_dict)."""
    def q_(a):  # quantize
        return a.astype(bf16).astype(np.float32) if use_bf16 else a

    prep = _prep(x, freqs_cis, mask_cache, wqkv, wo)
    cls = prep['cls']
    xT = prep['xT'].astype(np.float32)
    ropeA = np.concatenate([prep['ropeA'].astype(np.float32)] * B, axis=1)
    ropeB = np.concatenate([prep['ropeB'].astype(np.float32)] * B, axis=1)
    mask = np.asarray(mask_cache)[0, 0]

    dbg = {c: {} for c in range(NCORES)}
    a2a_ins = {m: [] for m in range(HPC)}  # m -> [core][8*128, 512]
    for c in range(NCORES):
        wT = prep['wTs'][c].astype(np.float32)
        qkvT = q_(wT.T @ xT)       # [768, TOK]  (psum f32, evict to bf16)
        sw = np.empty_like(qkvT[:5 * P])
        for f in range(5):
            blk = qkvT[f * P:(f + 1) * P]
            sw[f * P:(f + 1) * P] = q_(blk[[i ^ 1 for i in range(P)], :])
        roped = np.empty_like(qkvT[:5 * P])
        for f in range(5):
            blk = qkvT[f * P:(f + 1) * P]
            r1 = q_(blk * ropeA)
            r2 = q_(sw[f * P:(f + 1) * P] * ropeB)
            roped[f * P:(f + 1) * P] = q_(r1 + r2)
        qTs = [roped[h * P:(h + 1) * P] for h in range(HPC)]
        kTc = roped[4 * P:5 * P]
        vT = qkvT[5 * P:6 * P]     # [128 d, TOK], not roped
        for h in range(HPC):
            a2a_c = np.zeros((NCORES * P, 512), dtype=np.float32)
            for b in range(B):
                kTb = kTc[:, b * S:(b + 1) * S]
                vTb = vT[:, b * S:(b + 1) * S]
                qTb = qTs[h][:, b * S:(b + 1) * S]
                sT = kTb.T @ qTb               # [Sk, Sq] psum f32
                e = q_(np.exp(sT * SCALE))     # ACT exp -> bf16
                emask = e * mask.T             # mask multiply (exact 0/1)
                for qt in range(QT_N):
                    for kt in range(KT_N):
                        if cls[qt][kt] == 'skip':
                            emask[kt * P:(kt + 1) * P,
                                  qt * 512:(qt + 1) * 512] = 0
                D = emask.sum(axis=0)          # psum f32
                rec = 1.0 / D
                yTu = vTb @ emask
                y = q_(yTu * rec[None, :])
                for qt in range(QT_N):
                    r = b * QT_N + qt
                    a2a_c[r * P:(r + 1) * P] = y[:, qt * 512:(qt + 1) * 512]
            a2a_ins[h].append(a2a_c)

    out_full = np.zeros((B, S, DIM), dtype=np.float32)
    woT = prep['woT'].astype(np.float32)
    for c in range(NCORES):
        yfull = np.zeros((DIM, 512), dtype=np.float32)
        for m in range(HPC):
            for j in range(NCORES):
                dbi = m * NCORES + j
                yfull[dbi * P:(dbi + 1) * P] = \
                    a2a_ins[m][j][c * P:(c + 1) * P]
        o = yfull.T @ woT          # [512 tok, DIM] psum f32
        b, jj = c // QT_N, c % QT_N
        out_full[b, jj * 512:(jj + 1) * 512] = o
    return out_full, dbg

